# revision 13
# baseline (speedup 1.0000x reference)
"""Trainium2 Bass kernel for nn_DigitalTwinSimulator (2-layer LSTM + AR rollout).

Strategy: pure data parallel across 8 NeuronCores (batch 4096 -> 512/core).
Per core, state is kept feature-on-partitions / batch-on-free-dim.

Encode phase (t = 0..128): the two LSTM layers run in lockstep (layer 1 lags
layer 0 by one step) so all gate matmuls merge into K=128 block matmuls and
all elementwise ops are full 128-partition instructions. Biases + x
contribution enter through a K=5 matmul against [x_t; 1] (ones row built
host-side).

Matmul inputs (weights, h state, x) are fp16: same 1 cycle/row matmul
throughput as fp32r but ~2x cheaper LDWEIGHTS. Elementwise/activation data
stays fp32 (fp16 elementwise measured slower on ScalarE and no faster on
VectorE on this hardware).

AR phase (steps 128..187): pred feedback is algebraically folded:
  W0x @ pred = (W0x @ Wfc) @ h1 + W0x @ bfc
so the recurrence never materializes pred; the FC head runs off the critical
path purely for output. Layers run sequentially (inherent to AR). The AR
state keeps the encode layout ([h0; h1] stacked on partitions), so each
gate needs a single K=128 matmul (both source terms merged) instead of two
K=65 ones -- half the AR matmul traffic and no encode->AR transition.
Gate biases enter through the activation instruction's per-partition bias
operand; the FC output bias is added on the host.

Batch is split into 2 chunks of 256 columns that pipeline against each other
to hide the serial dependency chain latency.
"""
import os
import sys

for _p in ("/opt/trn_rl_repo", "/root/.axon_site/_ro/trn_rl_repo"):
    if os.path.isdir(_p) and _p not in sys.path:
        sys.path.append(_p)

import numpy as np

B, T, D, H, STEPS = 4096, 128, 4, 64, 60
NCORES = 8
BC = B // NCORES          # 512 batch rows per core
CH = 2                    # batch chunks per core (pipelined)
CW = BC // CH             # 256 columns per chunk

_cache = {}
TRACE = False
LAST = {}


def _build(T_, STEPS_):
    import concourse.bass as bass
    import concourse.tile as tile
    from concourse import bacc, mybir

    f32 = mybir.dt.float32
    f16 = mybir.dt.float16
    AF = mybir.ActivationFunctionType
    ALU = mybir.AluOpType

    nc = bacc.Bacc("TRN2", target_bir_lowering=False, debug=False,
                   num_devices=NCORES)

    xt_d = nc.dram_tensor("xt", (T_ + 1, 5, BC), f16, kind="ExternalInput")
    wencblk_d = nc.dram_tensor("wencblk", (128, 512), f16, kind="ExternalInput")
    wencx_d = nc.dram_tensor("wencx", (128, 512), f16, kind="ExternalInput")
    w1_d = nc.dram_tensor("w1", (128, 256), f16, kind="ExternalInput")
    w0_d = nc.dram_tensor("w0", (128, 256), f16, kind="ExternalInput")
    wfct_d = nc.dram_tensor("wfct", (128, 4), f16, kind="ExternalInput")
    bt_d = nc.dram_tensor("bt", (128, 4), f32, kind="ExternalInput")
    out_d = nc.dram_tensor("out", (STEPS_, 4, BC), f32, kind="ExternalOutput")

    with tile.TileContext(nc) as tc:
        with tc.tile_pool(name="const", bufs=1) as cpool, \
             tc.tile_pool(name="state", bufs=1) as spool, \
             tc.tile_pool(name="act", bufs=4) as apool, \
             tc.tile_pool(name="tmp", bufs=4) as tpool, \
             tc.tile_pool(name="psum", bufs=2, space="PSUM") as ppool:

            def dma_w(shape, src, tag, dt=f16):
                t = cpool.tile(list(shape), dt, tag=tag)
                nc.sync.dma_start(t[:], src.ap())
                return t

            wencblk = dma_w((128, 512), wencblk_d, "wencblk")
            wencx = dma_w((128, 512), wencx_d, "wencx")
            w1 = dma_w((128, 256), w1_d, "w1")
            w0 = dma_w((128, 256), w0_d, "w0")
            wfct = dma_w((128, 4), wfct_d, "wfct")
            bt = dma_w((128, 4), bt_d, "bt", dt=f32)

            # per-chunk persistent state (separate tiles so the two batch
            # chunks share no tiles -> no false cross-chunk dependencies)
            zsrc = spool.tile([128, BC], f16, tag="zsrc")
            nc.vector.memset(zsrc[:], 0.0)
            h_st, c_st, xins = [], [], []
            for ch in range(CH):
                h = spool.tile([128, CW], f16, tag=f"h_st{ch}")
                nc.vector.tensor_copy(h[:], zsrc[:, 0:CW])
                h_st.append(h)
                c = spool.tile([128, CW], f32, tag=f"c_st{ch}")
                nc.vector.memset(c[:], 0.0)
                c_st.append(c)
            # x staging: rows 5:128 stay zero => x matmul runs at K=128
            for _i in range(3):
                xt_t = spool.tile([128, BC], f16, tag=f"xin{_i}")
                nc.vector.tensor_copy(xt_t[:], zsrc[:])
                xins.append(xt_t)

            def encode_phase(p):
                xin = xins[p % 3]
                nc.sync.dma_start(xin[0:5, :], xt_d.ap()[p])
                for ch in range(CH):
                    cs = slice(ch * CW, (ch + 1) * CW)
                    h, c = h_st[ch], c_st[ch]
                    pif = ppool.tile([128, 2 * CW], f32, tag=f"pif{ch}")
                    pog = ppool.tile([128, 2 * CW], f32, tag=f"pog{ch}")
                    for (ps, col, q) in ((pif, 0, 0), (pif, 1, 1),
                                         (pog, 1, 2), (pog, 0, 3)):
                        dst = ps[:, col * CW:(col + 1) * CW]
                        nc.tensor.matmul(dst, wencblk[:, q * 128:(q + 1) * 128],
                                         h[:], start=True, stop=False)
                        nc.tensor.matmul(dst, wencx[:, q * 128:(q + 1) * 128],
                                         xin[:, cs], start=False, stop=True)
                    sif = apool.tile([128, 2 * CW], f32, tag=f"sif{ch}")
                    nc.scalar.activation(sif[:], pif[:], AF.Sigmoid)
                    sog = apool.tile([128, 2 * CW], f32, tag=f"sog{ch}")
                    nc.scalar.activation(sog[:, CW:], pog[:, CW:], AF.Tanh)
                    t2 = tpool.tile([128, CW], f32, tag=f"t2{ch}")
                    nc.vector.tensor_tensor(t2[:], sif[:, CW:], c[:], ALU.mult)
                    nc.scalar.activation(sog[:, 0:CW], pog[:, 0:CW], AF.Sigmoid)
                    t1 = tpool.tile([128, CW], f32, tag=f"t1{ch}")
                    nc.vector.tensor_tensor(t1[:], sif[:, 0:CW], sog[:, CW:], ALU.mult)
                    rows = slice(0, 64) if p == 0 else slice(0, 128)
                    nc.vector.tensor_tensor(c[rows, :], t1[rows, :], t2[rows, :], ALU.add)
                    thc = tpool.tile([128, CW], f32, tag=f"thc{ch}")
                    nc.scalar.activation(thc[:], c[:], AF.Tanh)
                    nc.vector.tensor_tensor(h[:], sog[:, 0:CW], thc[:], ALU.mult)

            for p in range(T_ + 1):
                encode_phase(p)

            # ---- AR phase: state stays in the encode layout ----
            # h_st[ch] rows 0:64 = h0, rows 64:128 = h1 (same for c_st).
            # The cell for layer L runs entirely on its partition half
            # (L0 -> rows 0:64, L1 -> rows 64:128): gates, activations and
            # elementwise all live on those partitions, so the h/c updates
            # write straight back into the stacked state with no crossing.
            def ar_cell(ch, w, rows):
                pq_if = ppool.tile([128, 2 * CW], f32, tag=f"pif{ch}")
                pq_og = ppool.tile([128, 2 * CW], f32, tag=f"pog{ch}")
                for (ps, col, j) in ((pq_if, 0, 0), (pq_if, 1, 1),
                                     (pq_og, 1, 3), (pq_og, 0, 2)):
                    dst = ps[rows, col * CW:(col + 1) * CW]
                    nc.tensor.matmul(dst, w[:, j * 64:(j + 1) * 64],
                                     h_st[ch][:], start=True, stop=True)
                sif = apool.tile([128, 2 * CW], f32, tag=f"sif{ch}")
                nc.scalar.activation(sif[rows, CW:], pq_if[rows, CW:],
                                     AF.Sigmoid, bias=bt[rows, 1:2])
                t2 = tpool.tile([128, CW], f32, tag=f"t2{ch}")
                nc.vector.tensor_tensor(t2[rows, :], sif[rows, CW:],
                                        c_st[ch][rows, :], ALU.mult)
                gt = tpool.tile([128, CW], f32, tag=f"gt{ch}")
                nc.scalar.activation(gt[rows, :], pq_og[rows, CW:],
                                     AF.Tanh, bias=bt[rows, 3:4])
                nc.scalar.activation(sif[rows, 0:CW], pq_if[rows, 0:CW],
                                     AF.Sigmoid, bias=bt[rows, 0:1])
                t1 = tpool.tile([128, CW], f32, tag=f"t1{ch}")
                nc.vector.tensor_tensor(t1[rows, :], sif[rows, 0:CW],
                                        gt[rows, :], ALU.mult)
                so = tpool.tile([128, CW], f32, tag=f"so{ch}")
                nc.scalar.activation(so[rows, :], pq_og[rows, 0:CW],
                                     AF.Sigmoid, bias=bt[rows, 2:3])
                nc.vector.tensor_tensor(c_st[ch][rows, :], t1[rows, :],
                                        t2[rows, :], ALU.add)
                thc = tpool.tile([128, CW], f32, tag=f"thc{ch}")
                nc.scalar.activation(thc[rows, :], c_st[ch][rows, :], AF.Tanh)
                nc.vector.tensor_tensor(h_st[ch][rows, :], so[rows, :],
                                        thc[rows, :], ALU.mult)
                return pq_if

            ROWS1 = slice(64, 128)
            ROWS0 = slice(0, 64)
            for s in range(T_, T_ + STEPS_):
                last = s == T_ + STEPS_ - 1
                for ch in range(CH):
                    cs = slice(ch * CW, (ch + 1) * CW)
                    p1if = ar_cell(ch, w1, ROWS1)
                    # pred = Wfc @ h1 (bias added host-side); reuses the dead
                    # rows 0:4 of p1if after its gate rows were consumed
                    pp = p1if[0:4, 0:CW]
                    nc.tensor.matmul(pp, wfct[ROWS1, :], h_st[ch][ROWS1, :],
                                     start=True, stop=True)
                    po = tpool.tile([4, CW], f32, tag=f"po{ch}")
                    nc.vector.tensor_copy(po[:], pp)
                    nc.sync.dma_start(out_d.ap()[s - T_, :, cs], po[:])
                    if not last:
                        ar_cell(ch, w0, ROWS0)

    nc.compile()
    return nc


def _prep_inputs(x, Wih0, Whh0, bih0, bhh0, Wih1, Whh1, bih1, bhh1, Wfc, bfc,
                 T_, STEPS_):
    """Host-side layout prep. Returns (shared_weight_arrays, per_core_xt)."""
    f = np.float32
    h16 = np.float16
    x = np.asarray(x, f)
    Wih0, Whh0 = np.asarray(Wih0, f), np.asarray(Whh0, f)
    Wih1, Whh1 = np.asarray(Wih1, f), np.asarray(Whh1, f)
    Wfc = np.asarray(Wfc, f)
    b0 = np.asarray(bih0, f) + np.asarray(bhh0, f)   # [4H]
    b1 = np.asarray(bih1, f) + np.asarray(bhh1, f)
    bfc = np.asarray(bfc, f)

    def gate(Wm, q):
        return Wm[q * H:(q + 1) * H]

    # encode block lhsT [128k, 4 gates x 128m]
    wencblk = np.zeros((128, 512), f)
    wencx = np.zeros((128, 512), f)
    for q in range(4):
        blk = np.zeros((128, 128), f)
        blk[0:64, 0:64] = gate(Whh0, q).T
        blk[0:64, 64:128] = gate(Wih1, q).T
        blk[64:128, 64:128] = gate(Whh1, q).T
        wencblk[:, q * 128:(q + 1) * 128] = blk
        wx = np.zeros((5, 128), f)
        wx[0:4, 0:64] = gate(Wih0, q).T
        wx[4, 0:64] = gate(b0[:, None], q)[:, 0]
        wx[4, 64:128] = gate(b1[:, None], q)[:, 0]
        wencx[0:5, q * 128:(q + 1) * 128] = wx

    # AR weights: lhsT col blocks = gates in order (i, f, o, g), M=64 each.
    # K rows 0:64 multiply h0, rows 64:128 multiply h1 (encode state layout).
    Wcomb = Wih0 @ Wfc          # [4H, 64]
    b0p = b0 + Wih0 @ bfc
    colq = (0, 1, 3, 2)
    w1 = np.zeros((128, 256), f)
    w0 = np.zeros((128, 256), f)
    bt = np.zeros((128, 4), f)
    for j, q in enumerate(colq):
        mc = slice(j * 64, (j + 1) * 64)
        w1[0:64, mc] = gate(Wih1, q).T
        w1[64:128, mc] = gate(Whh1, q).T
        w0[0:64, mc] = gate(Whh0, q).T
        w0[64:128, mc] = gate(Wcomb, q).T
        bt[0:64, j] = gate(b0p[:, None], q)[:, 0]
        bt[64:128, j] = gate(b1[:, None], q)[:, 0]
    wfct = np.zeros((128, 4), f)
    wfct[64:128] = Wfc.T

    shared = dict(wencblk=wencblk.astype(h16), wencx=wencx.astype(h16),
                  w1=w1.astype(h16), w0=w0.astype(h16),
                  wfct=wfct.astype(h16), bt=bt)

    # per-core x-tilde: [T+1, 5, BC]; row 4 = ones; step T duplicates x_{T-1}
    xts = []
    for c in range(NCORES):
        xs = x[c * BC:(c + 1) * BC, :T_, :]          # [BC, T_, D]
        xt = np.ones((T_ + 1, 5, BC), f)
        xt[:T_, 0:4, :] = np.transpose(xs, (1, 2, 0))
        xt[T_, 0:4, :] = xs[:, T_ - 1, :].T
        xts.append(xt.astype(h16))
    return shared, xts


def kernel(**inputs):
    return _run(T, STEPS, **inputs)


def _run(T_, STEPS_, x, Wih0, Whh0, bih0, bhh0, Wih1, Whh1, bih1, bhh1,
         Wfc, bfc):
    from concourse.bass_utils import run_bass_kernel_spmd

    key = (T_, STEPS_)
    if key not in _cache:
        _cache[key] = _build(T_, STEPS_)
    nc = _cache[key]

    shared, xts = _prep_inputs(x, Wih0, Whh0, bih0, bhh0, Wih1, Whh1,
                               bih1, bhh1, Wfc, bfc, T_, STEPS_)
    in_maps = [{**shared, "xt": xts[c]} for c in range(NCORES)]
    res = run_bass_kernel_spmd(nc, in_maps, core_ids=list(range(NCORES)),
                               trace=TRACE)
    LAST["exec_time_ns"] = res.exec_time_ns
    LAST["res"] = res
    out = np.empty((B, STEPS_, 4), np.float32)
    for c in range(NCORES):
        # res: [STEPS, 4, BC] -> [BC, STEPS, 4]
        out[c * BC:(c + 1) * BC] = np.transpose(res.results[c]["out"], (2, 0, 1))
    out += np.asarray(bfc, np.float32)      # FC bias folded out of the device
    return out


# revision 17
# speedup vs baseline: 1.0043x; 1.0043x over previous
"""Trainium2 Bass kernel for nn_DigitalTwinSimulator (2-layer LSTM + AR rollout).

Strategy: pure data parallel across 8 NeuronCores (batch 4096 -> 512/core).
Per core, state is kept feature-on-partitions / batch-on-free-dim.

Encode phase (t = 0..128): the two LSTM layers run in lockstep (layer 1 lags
layer 0 by one step) so all gate matmuls merge into K=128 block matmuls and
all elementwise ops are full 128-partition instructions. Biases + x
contribution enter through a K=5 matmul against [x_t; 1] (ones row built
host-side).

Matmul inputs (weights, h state, x) are fp16: same 1 cycle/row matmul
throughput as fp32r but ~2x cheaper LDWEIGHTS. Elementwise/activation data
stays fp32 (fp16 elementwise measured slower on ScalarE and no faster on
VectorE on this hardware).

AR phase (steps 128..187): pred feedback is algebraically folded:
  W0x @ pred = (W0x @ Wfc) @ h1 + W0x @ bfc
so the recurrence never materializes pred; the FC head runs off the critical
path purely for output. Layers run sequentially (inherent to AR). The AR
state keeps the encode layout ([h0; h1] stacked on partitions), so each
gate needs a single K=128 matmul (both source terms merged) instead of two
K=65 ones -- half the AR matmul traffic and no encode->AR transition.
Gate biases enter through the activation instruction's per-partition bias
operand; the FC output bias is added on the host.

Batch is split into 2 chunks of 256 columns that pipeline against each other
to hide the serial dependency chain latency.
"""
import os
import sys

for _p in ("/opt/trn_rl_repo", "/root/.axon_site/_ro/trn_rl_repo"):
    if os.path.isdir(_p) and _p not in sys.path:
        sys.path.append(_p)

import numpy as np

B, T, D, H, STEPS = 4096, 128, 4, 64, 60
NCORES = 8
BC = B // NCORES          # 512 batch rows per core
CH = 2                    # batch chunks per core (pipelined)
CW = BC // CH             # 256 columns per chunk

_cache = {}
TRACE = False
LAST = {}


def _build(T_, STEPS_):
    import concourse.bass as bass
    import concourse.tile as tile
    from concourse import bacc, mybir

    f32 = mybir.dt.float32
    f16 = mybir.dt.float16
    AF = mybir.ActivationFunctionType
    ALU = mybir.AluOpType

    nc = bacc.Bacc("TRN2", target_bir_lowering=False, debug=False,
                   num_devices=NCORES)

    xt_d = nc.dram_tensor("xt", (T_ + 1, 5, BC), f16, kind="ExternalInput")
    wencblk_d = nc.dram_tensor("wencblk", (128, 512), f16, kind="ExternalInput")
    wencx_d = nc.dram_tensor("wencx", (128, 512), f16, kind="ExternalInput")
    w1_d = nc.dram_tensor("w1", (128, 256), f16, kind="ExternalInput")
    w0_d = nc.dram_tensor("w0", (128, 256), f16, kind="ExternalInput")
    wfct_d = nc.dram_tensor("wfct", (128, 4), f16, kind="ExternalInput")
    bbif_d = nc.dram_tensor("bbif", (2, 128), f16, kind="ExternalInput")
    bbog_d = nc.dram_tensor("bbog", (2, 128), f16, kind="ExternalInput")
    bsel_d = nc.dram_tensor("bsel", (2, 512), f16, kind="ExternalInput")
    out_d = nc.dram_tensor("out", (STEPS_, 4, BC), f32, kind="ExternalOutput")

    with tile.TileContext(nc) as tc:
        with tc.tile_pool(name="const", bufs=1) as cpool, \
             tc.tile_pool(name="state", bufs=1) as spool, \
             tc.tile_pool(name="act", bufs=4) as apool, \
             tc.tile_pool(name="tmp", bufs=4) as tpool, \
             tc.tile_pool(name="psum", bufs=2, space="PSUM") as ppool:

            def dma_w(shape, src, tag, dt=f16):
                t = cpool.tile(list(shape), dt, tag=tag)
                nc.sync.dma_start(t[:], src.ap())
                return t

            wencblk = dma_w((128, 512), wencblk_d, "wencblk")
            wencx = dma_w((128, 512), wencx_d, "wencx")
            w1 = dma_w((128, 256), w1_d, "w1")
            w0 = dma_w((128, 256), w0_d, "w0")
            wfct = dma_w((128, 4), wfct_d, "wfct")
            bbif = dma_w((2, 128), bbif_d, "bbif")
            bbog = dma_w((2, 128), bbog_d, "bbog")
            bsel = dma_w((2, 512), bsel_d, "bsel")

            # per-chunk persistent state (separate tiles so the two batch
            # chunks share no tiles -> no false cross-chunk dependencies)
            zsrc = spool.tile([128, BC], f16, tag="zsrc")
            nc.vector.memset(zsrc[:], 0.0)
            h_st, c_st, xins = [], [], []
            for ch in range(CH):
                h = spool.tile([128, CW], f16, tag=f"h_st{ch}")
                nc.vector.tensor_copy(h[:], zsrc[:, 0:CW])
                h_st.append(h)
                c = spool.tile([128, CW], f32, tag=f"c_st{ch}")
                nc.vector.memset(c[:], 0.0)
                c_st.append(c)
            # x staging: rows 5:128 stay zero => x matmul runs at K=128
            for _i in range(3):
                xt_t = spool.tile([128, BC], f16, tag=f"xin{_i}")
                nc.vector.tensor_copy(xt_t[:], zsrc[:])
                xins.append(xt_t)

            def encode_phase(p):
                xin = xins[p % 3]
                nc.sync.dma_start(xin[0:5, :], xt_d.ap()[p])
                for ch in range(CH):
                    cs = slice(ch * CW, (ch + 1) * CW)
                    h, c = h_st[ch], c_st[ch]
                    pif = ppool.tile([128, 2 * CW], f32, tag=f"pif{ch}")
                    pog = ppool.tile([128, 2 * CW], f32, tag=f"pog{ch}")
                    for (ps, col, q) in ((pif, 0, 0), (pif, 1, 1),
                                         (pog, 1, 2), (pog, 0, 3)):
                        dst = ps[:, col * CW:(col + 1) * CW]
                        nc.tensor.matmul(dst, wencblk[:, q * 128:(q + 1) * 128],
                                         h[:], start=True, stop=False)
                        nc.tensor.matmul(dst, wencx[:, q * 128:(q + 1) * 128],
                                         xin[:, cs], start=False, stop=True)
                    sif = apool.tile([128, 2 * CW], f32, tag=f"sif{ch}")
                    nc.scalar.activation(sif[:], pif[:], AF.Sigmoid)
                    sog = apool.tile([128, 2 * CW], f32, tag=f"sog{ch}")
                    nc.scalar.activation(sog[:, CW:], pog[:, CW:], AF.Tanh)
                    t2 = tpool.tile([128, CW], f32, tag=f"t2{ch}")
                    nc.vector.tensor_tensor(t2[:], sif[:, CW:], c[:], ALU.mult)
                    nc.scalar.activation(sog[:, 0:CW], pog[:, 0:CW], AF.Sigmoid)
                    t1 = tpool.tile([128, CW], f32, tag=f"t1{ch}")
                    nc.vector.tensor_tensor(t1[:], sif[:, 0:CW], sog[:, CW:], ALU.mult)
                    rows = slice(0, 64) if p == 0 else slice(0, 128)
                    nc.vector.tensor_tensor(c[rows, :], t1[rows, :], t2[rows, :], ALU.add)
                    thc = tpool.tile([128, CW], f32, tag=f"thc{ch}")
                    nc.scalar.activation(thc[:], c[:], AF.Tanh)
                    nc.vector.tensor_tensor(h[:], sog[:, 0:CW], thc[:], ALU.mult)

            for p in range(T_ + 1):
                encode_phase(p)

            # ---- AR phase: state stays in the encode layout ----
            # h_st[ch] rows 0:64 = h0, rows 64:128 = h1 (same for c_st).
            # The cell for layer L runs entirely on its partition half
            # (L0 -> rows 0:64, L1 -> rows 64:128): gates, activations and
            # elementwise all live on those partitions, so the h/c updates
            # write straight back into the stacked state with no crossing.
            def ar_cell(ch, w, rows):
                lc = slice(rows.start, rows.start + 64)   # layer col in bb*
                pq_if = ppool.tile([128, 2 * CW], f32, tag=f"pif{ch}")
                pq_og = ppool.tile([128, 2 * CW], f32, tag=f"pog{ch}")
                # K=2 bias matmul: rank-2 product gives a bias varying by
                # partition (feature) AND by gate column block; prefetchable
                # since it only reads constants
                nc.tensor.matmul(pq_if[rows, :], bbif[:, lc], bsel[:],
                                 start=True, stop=False, skip_group_check=True)
                nc.tensor.matmul(pq_og[rows, :], bbog[:, lc], bsel[:],
                                 start=True, stop=False, skip_group_check=True)
                for (ps, col, j) in ((pq_if, 0, 0), (pq_if, 1, 1),
                                     (pq_og, 1, 3), (pq_og, 0, 2)):
                    dst = ps[rows, col * CW:(col + 1) * CW]
                    nc.tensor.matmul(dst, w[:, j * 64:(j + 1) * 64],
                                     h_st[ch][:], start=False, stop=True,
                                     skip_group_check=True)
                sif = apool.tile([128, 2 * CW], f32, tag=f"sif{ch}")
                nc.scalar.activation(sif[rows, :], pq_if[rows, :], AF.Sigmoid)
                t2 = tpool.tile([128, CW], f32, tag=f"t2{ch}")
                nc.vector.tensor_tensor(t2[rows, :], sif[rows, CW:],
                                        c_st[ch][rows, :], ALU.mult)
                gt = tpool.tile([128, CW], f32, tag=f"gt{ch}")
                nc.scalar.activation(gt[rows, :], pq_og[rows, CW:], AF.Tanh)
                t1 = tpool.tile([128, CW], f32, tag=f"t1{ch}")
                nc.vector.tensor_tensor(t1[rows, :], sif[rows, 0:CW],
                                        gt[rows, :], ALU.mult)
                so = tpool.tile([128, CW], f32, tag=f"so{ch}")
                nc.scalar.activation(so[rows, :], pq_og[rows, 0:CW], AF.Sigmoid)
                nc.vector.tensor_tensor(c_st[ch][rows, :], t1[rows, :],
                                        t2[rows, :], ALU.add)
                thc = tpool.tile([128, CW], f32, tag=f"thc{ch}")
                nc.scalar.activation(thc[rows, :], c_st[ch][rows, :], AF.Tanh)
                nc.vector.tensor_tensor(h_st[ch][rows, :], so[rows, :],
                                        thc[rows, :], ALU.mult)
                return pq_if

            ROWS1 = slice(64, 128)
            ROWS0 = slice(0, 64)
            for s in range(T_, T_ + STEPS_):
                last = s == T_ + STEPS_ - 1
                for ch in range(CH):
                    cs = slice(ch * CW, (ch + 1) * CW)
                    p1if = ar_cell(ch, w1, ROWS1)
                    # pred = Wfc @ h1 (bias added host-side); reuses the dead
                    # rows 0:4 of p1if after its gate rows were consumed
                    pp = p1if[0:4, 0:CW]
                    nc.tensor.matmul(pp, wfct[ROWS1, :], h_st[ch][ROWS1, :],
                                     start=True, stop=True)
                    po = tpool.tile([4, CW], f32, tag=f"po{ch}")
                    nc.vector.tensor_copy(po[:], pp)
                    nc.sync.dma_start(out_d.ap()[s - T_, :, cs], po[:])
                    if not last:
                        ar_cell(ch, w0, ROWS0)

    nc.compile()
    return nc


def _prep_inputs(x, Wih0, Whh0, bih0, bhh0, Wih1, Whh1, bih1, bhh1, Wfc, bfc,
                 T_, STEPS_):
    """Host-side layout prep. Returns (shared_weight_arrays, per_core_xt)."""
    f = np.float32
    h16 = np.float16
    x = np.asarray(x, f)
    Wih0, Whh0 = np.asarray(Wih0, f), np.asarray(Whh0, f)
    Wih1, Whh1 = np.asarray(Wih1, f), np.asarray(Whh1, f)
    Wfc = np.asarray(Wfc, f)
    b0 = np.asarray(bih0, f) + np.asarray(bhh0, f)   # [4H]
    b1 = np.asarray(bih1, f) + np.asarray(bhh1, f)
    bfc = np.asarray(bfc, f)

    def gate(Wm, q):
        return Wm[q * H:(q + 1) * H]

    # encode block lhsT [128k, 4 gates x 128m]
    wencblk = np.zeros((128, 512), f)
    wencx = np.zeros((128, 512), f)
    for q in range(4):
        blk = np.zeros((128, 128), f)
        blk[0:64, 0:64] = gate(Whh0, q).T
        blk[0:64, 64:128] = gate(Wih1, q).T
        blk[64:128, 64:128] = gate(Whh1, q).T
        wencblk[:, q * 128:(q + 1) * 128] = blk
        wx = np.zeros((5, 128), f)
        wx[0:4, 0:64] = gate(Wih0, q).T
        wx[4, 0:64] = gate(b0[:, None], q)[:, 0]
        wx[4, 64:128] = gate(b1[:, None], q)[:, 0]
        wencx[0:5, q * 128:(q + 1) * 128] = wx

    # AR weights: lhsT col blocks = gates in order (i, f, o, g), M=64 each.
    # K rows 0:64 multiply h0, rows 64:128 multiply h1 (encode state layout).
    Wcomb = Wih0 @ Wfc          # [4H, 64]
    b0p = b0 + Wih0 @ bfc
    colq = (0, 1, 3, 2)
    w1 = np.zeros((128, 256), f)
    w0 = np.zeros((128, 256), f)
    for j, q in enumerate(colq):
        mc = slice(j * 64, (j + 1) * 64)
        w1[0:64, mc] = gate(Wih1, q).T
        w1[64:128, mc] = gate(Whh1, q).T
        w0[0:64, mc] = gate(Whh0, q).T
        w0[64:128, mc] = gate(Wcomb, q).T
    wfct = np.zeros((128, 4), f)
    wfct[64:128] = Wfc.T
    # K=2 bias-matmul operands: lhsT rows = the two gate blocks of each PSUM
    # tile, cols 0:64 = L0 biases (b0p), 64:128 = L1 biases (b1);
    # bsel rows select which column block the bias lands in
    bbif = np.zeros((2, 128), f)   # pif blocks: i (col 0), f (col 1)
    bbog = np.zeros((2, 128), f)   # pog blocks: o (col 0), g (col 1)
    for r, q in ((0, 0), (1, 1)):
        bbif[r, 0:64] = gate(b0p[:, None], q)[:, 0]
        bbif[r, 64:128] = gate(b1[:, None], q)[:, 0]
    for r, q in ((0, 3), (1, 2)):
        bbog[r, 0:64] = gate(b0p[:, None], q)[:, 0]
        bbog[r, 64:128] = gate(b1[:, None], q)[:, 0]
    bsel = np.zeros((2, 512), f)
    bsel[0, 0:256] = 1.0
    bsel[1, 256:512] = 1.0

    shared = dict(wencblk=wencblk.astype(h16), wencx=wencx.astype(h16),
                  w1=w1.astype(h16), w0=w0.astype(h16),
                  wfct=wfct.astype(h16), bbif=bbif.astype(h16),
                  bbog=bbog.astype(h16), bsel=bsel.astype(h16))

    # per-core x-tilde: [T+1, 5, BC]; row 4 = ones; step T duplicates x_{T-1}
    xts = []
    for c in range(NCORES):
        xs = x[c * BC:(c + 1) * BC, :T_, :]          # [BC, T_, D]
        xt = np.ones((T_ + 1, 5, BC), f)
        xt[:T_, 0:4, :] = np.transpose(xs, (1, 2, 0))
        xt[T_, 0:4, :] = xs[:, T_ - 1, :].T
        xts.append(xt.astype(h16))
    return shared, xts


def kernel(**inputs):
    return _run(T, STEPS, **inputs)


def _run(T_, STEPS_, x, Wih0, Whh0, bih0, bhh0, Wih1, Whh1, bih1, bhh1,
         Wfc, bfc):
    from concourse.bass_utils import run_bass_kernel_spmd

    key = (T_, STEPS_)
    if key not in _cache:
        _cache[key] = _build(T_, STEPS_)
    nc = _cache[key]

    shared, xts = _prep_inputs(x, Wih0, Whh0, bih0, bhh0, Wih1, Whh1,
                               bih1, bhh1, Wfc, bfc, T_, STEPS_)
    in_maps = [{**shared, "xt": xts[c]} for c in range(NCORES)]
    res = run_bass_kernel_spmd(nc, in_maps, core_ids=list(range(NCORES)),
                               trace=TRACE)
    LAST["exec_time_ns"] = res.exec_time_ns
    LAST["res"] = res
    out = np.empty((B, STEPS_, 4), np.float32)
    for c in range(NCORES):
        # res: [STEPS, 4, BC] -> [BC, STEPS, 4]
        out[c * BC:(c + 1) * BC] = np.transpose(res.results[c]["out"], (2, 0, 1))
    out += np.asarray(bfc, np.float32)      # FC bias folded out of the device
    return out


# revision 20
# speedup vs baseline: 1.0053x; 1.0010x over previous
"""Trainium2 Bass kernel for nn_DigitalTwinSimulator (2-layer LSTM + AR rollout).

Strategy: pure data parallel across 8 NeuronCores (batch 4096 -> 512/core).
Per core, state is kept feature-on-partitions / batch-on-free-dim.

Encode phase (t = 0..128): the two LSTM layers run in lockstep (layer 1 lags
layer 0 by one step) so all gate matmuls merge into K=128 block matmuls and
all elementwise ops are full 128-partition instructions. Biases + x
contribution enter through a K=5 matmul against [x_t; 1] (ones row built
host-side).

Matmul inputs (weights, h state, x) are fp16: same 1 cycle/row matmul
throughput as fp32r but ~2x cheaper LDWEIGHTS. Elementwise/activation data
stays fp32 (fp16 elementwise measured slower on ScalarE and no faster on
VectorE on this hardware).

AR phase (steps 128..187): pred feedback is algebraically folded:
  W0x @ pred = (W0x @ Wfc) @ h1 + W0x @ bfc
so the recurrence never materializes pred; the FC head runs off the critical
path purely for output. Layers run sequentially (inherent to AR); biases ride
in a 65th row of the h1 tile (ones row).

Batch is split into 2 chunks of 256 columns that pipeline against each other
to hide the serial dependency chain latency.
"""
import os
import sys

for _p in ("/opt/trn_rl_repo", "/root/.axon_site/_ro/trn_rl_repo"):
    if os.path.isdir(_p) and _p not in sys.path:
        sys.path.append(_p)

import numpy as np

B, T, D, H, STEPS = 4096, 128, 4, 64, 60
NCORES = 8
BC = B // NCORES          # 512 batch rows per core
CH = 2                    # batch chunks per core (pipelined)
CW = BC // CH             # 256 columns per chunk

_cache = {}
TRACE = False
LAST = {}


def _build(T_, STEPS_):
    import concourse.bass as bass
    import concourse.tile as tile
    from concourse import bacc, mybir

    f32 = mybir.dt.float32
    f16 = mybir.dt.float16
    AF = mybir.ActivationFunctionType
    ALU = mybir.AluOpType

    nc = bacc.Bacc("TRN2", target_bir_lowering=False, debug=False,
                   num_devices=NCORES)

    xt_d = nc.dram_tensor("xt", (T_ + 1, 5, BC), f16, kind="ExternalInput")
    wencblk_d = nc.dram_tensor("wencblk", (128, 512), f16, kind="ExternalInput")
    wencx_d = nc.dram_tensor("wencx", (128, 512), f16, kind="ExternalInput")
    w1h0_d = nc.dram_tensor("w1h0", (65, 256), f16, kind="ExternalInput")
    w1h1b_d = nc.dram_tensor("w1h1b", (65, 256), f16, kind="ExternalInput")
    w0h0_d = nc.dram_tensor("w0h0", (65, 256), f16, kind="ExternalInput")
    w0h1b_d = nc.dram_tensor("w0h1b", (65, 256), f16, kind="ExternalInput")
    wfcb_d = nc.dram_tensor("wfcb", (65, 4), f16, kind="ExternalInput")
    ones_d = nc.dram_tensor("ones_row", (1, BC), f16, kind="ExternalInput")
    out_d = nc.dram_tensor("out", (STEPS_, 4, BC), f32, kind="ExternalOutput")

    with tile.TileContext(nc) as tc:
        with tc.tile_pool(name="const", bufs=1) as cpool, \
             tc.tile_pool(name="state", bufs=1) as spool, \
             tc.tile_pool(name="act", bufs=6) as apool, \
             tc.tile_pool(name="tmp", bufs=6) as tpool, \
             tc.tile_pool(name="psum", bufs=2, space="PSUM") as ppool:

            def dma_w(shape, src, tag):
                t = cpool.tile(list(shape), f16, tag=tag)
                nc.sync.dma_start(t[:], src.ap())
                return t

            wencblk = dma_w((128, 512), wencblk_d, "wencblk")
            wencx = dma_w((128, 512), wencx_d, "wencx")
            w1h0 = dma_w((65, 256), w1h0_d, "w1h0")
            w1h1b = dma_w((65, 256), w1h1b_d, "w1h1b")
            w0h0 = dma_w((65, 256), w0h0_d, "w0h0")
            w0h1b = dma_w((65, 256), w0h1b_d, "w0h1b")
            wfcb = dma_w((65, 4), wfcb_d, "wfcb")

            # per-chunk persistent state (separate tiles so the two batch
            # chunks share no tiles -> no false cross-chunk dependencies)
            zsrc = spool.tile([128, BC], f16, tag="zsrc")
            nc.vector.memset(zsrc[:], 0.0)
            h_st, c_st, xins = [], [], []
            for ch in range(CH):
                h = spool.tile([128, CW], f16, tag=f"h_st{ch}")
                nc.vector.tensor_copy(h[:], zsrc[:, 0:CW])
                h_st.append(h)
                c = spool.tile([128, CW], f32, tag=f"c_st{ch}")
                nc.vector.memset(c[:], 0.0)
                c_st.append(c)
            # x staging: rows 5:128 stay zero => x matmul runs at K=128
            for _i in range(3):
                xt_t = spool.tile([128, BC], f16, tag=f"xin{_i}")
                nc.vector.tensor_copy(xt_t[:], zsrc[:])
                xins.append(xt_t)

            # x DMA runs 2 steps ahead of its consumer so it never sits on
            # the per-step critical path (3 staging buffers rotate)
            nc.sync.dma_start(xins[0][0:5, :], xt_d.ap()[0])
            nc.sync.dma_start(xins[1][0:5, :], xt_d.ap()[1])

            def encode_phase(p):
                xin = xins[p % 3]
                if p + 2 <= T_:
                    nc.sync.dma_start(xins[(p + 2) % 3][0:5, :],
                                      xt_d.ap()[p + 2])
                for ch in range(CH):
                    cs = slice(ch * CW, (ch + 1) * CW)
                    h, c = h_st[ch], c_st[ch]
                    pif = ppool.tile([128, 2 * CW], f32, tag=f"pif{ch}")
                    pog = ppool.tile([128, 2 * CW], f32, tag=f"pog{ch}")
                    for (ps, col, q) in ((pif, 0, 0), (pif, 1, 1),
                                         (pog, 1, 2), (pog, 0, 3)):
                        dst = ps[:, col * CW:(col + 1) * CW]
                        nc.tensor.matmul(dst, wencblk[:, q * 128:(q + 1) * 128],
                                         h[:], start=True, stop=False)
                        nc.tensor.matmul(dst, wencx[:, q * 128:(q + 1) * 128],
                                         xin[:, cs], start=False, stop=True)
                    sif = apool.tile([128, 2 * CW], f32, tag=f"sif{ch}")
                    nc.scalar.activation(sif[:], pif[:], AF.Sigmoid)
                    sog = apool.tile([128, 2 * CW], f32, tag=f"sog{ch}")
                    nc.scalar.activation(sog[:, CW:], pog[:, CW:], AF.Tanh)
                    t2 = tpool.tile([128, CW], f32, tag=f"t2{ch}")
                    nc.vector.tensor_tensor(t2[:], sif[:, CW:], c[:], ALU.mult)
                    nc.scalar.activation(sog[:, 0:CW], pog[:, 0:CW], AF.Sigmoid)
                    t1 = tpool.tile([128, CW], f32, tag=f"t1{ch}")
                    nc.vector.tensor_tensor(t1[:], sif[:, 0:CW], sog[:, CW:], ALU.mult)
                    rows = slice(0, 64) if p == 0 else slice(0, 128)
                    nc.vector.tensor_tensor(c[rows, :], t1[rows, :], t2[rows, :], ALU.add)
                    thc = tpool.tile([128, CW], f32, tag=f"thc{ch}")
                    nc.scalar.activation(thc[:], c[:], AF.Tanh)
                    nc.vector.tensor_tensor(h[:], sog[:, 0:CW], thc[:], ALU.mult)

            for p in range(T_ + 1):
                encode_phase(p)

            # ---- transition to AR layout (per-chunk tiles) ----
            h0t, h1b, c0t, c1t = [], [], [], []
            for ch in range(CH):
                cs = slice(ch * CW, (ch + 1) * CW)
                a = spool.tile([65, CW], f16, tag=f"h0t{ch}")
                nc.sync.dma_start(a[0:64, :], h_st[ch][0:64, :])
                nc.sync.dma_start(a[64:65, :], ones_d.ap()[0:1, cs])
                h0t.append(a)
                b = spool.tile([65, CW], f16, tag=f"h1b{ch}")
                nc.sync.dma_start(b[0:64, :], h_st[ch][64:128, :])
                nc.sync.dma_start(b[64:65, :], ones_d.ap()[0:1, cs])
                h1b.append(b)
                c0 = spool.tile([64, CW], f32, tag=f"c0t{ch}")
                nc.sync.dma_start(c0[:], c_st[ch][0:64, :])
                c0t.append(c0)
                c1 = spool.tile([64, CW], f32, tag=f"c1t{ch}")
                nc.sync.dma_start(c1[:], c_st[ch][64:128, :])
                c1t.append(c1)

            def ar_mms(ch, wh0, wh1b):
                pq_if = ppool.tile([64, 2 * CW], f32, tag=f"pif{ch}")
                pq_og = ppool.tile([64, 2 * CW], f32, tag=f"pog{ch}")
                for (ps, col, g) in ((pq_if, 0, 0), (pq_if, 1, 1),
                                     (pq_og, 1, 3), (pq_og, 0, 2)):
                    dst = ps[:, col * CW:(col + 1) * CW]
                    nc.tensor.matmul(dst, wh0[:, g * 64:(g + 1) * 64],
                                     h0t[ch][:], start=True, stop=False)
                    nc.tensor.matmul(dst, wh1b[:, g * 64:(g + 1) * 64],
                                     h1b[ch][:], start=False, stop=True)
                return pq_if, pq_og

            def ar_tail(ch, pq_if, pq_og, ct, hout_t, hout_rows):
                sif = apool.tile([64, 2 * CW], f32, tag=f"sif{ch}")
                nc.scalar.activation(sif[:], pq_if[:], AF.Sigmoid)
                gt = tpool.tile([64, CW], f32, tag=f"gt{ch}")
                nc.scalar.activation(gt[:], pq_og[:, CW:], AF.Tanh)
                t2 = tpool.tile([64, CW], f32, tag=f"t2{ch}")
                nc.vector.tensor_tensor(t2[:], sif[:, CW:], ct[:], ALU.mult)
                so = tpool.tile([64, CW], f32, tag=f"so{ch}")
                nc.scalar.activation(so[:], pq_og[:, 0:CW], AF.Sigmoid)
                t1 = tpool.tile([64, CW], f32, tag=f"t1{ch}")
                nc.vector.tensor_tensor(t1[:], sif[:, 0:CW], gt[:], ALU.mult)
                nc.vector.tensor_tensor(ct[:], t1[:], t2[:], ALU.add)
                thc = tpool.tile([64, CW], f32, tag=f"thc{ch}")
                nc.scalar.activation(thc[:], ct[:], AF.Tanh)
                nc.vector.tensor_tensor(hout_t[hout_rows, :], so[:], thc[:], ALU.mult)

            for s in range(T_, T_ + STEPS_):
                last = s == T_ + STEPS_ - 1
                for ch in range(CH):
                    cs = slice(ch * CW, (ch + 1) * CW)
                    p1if, p1og = ar_mms(ch, w1h0, w1h1b)
                    ar_tail(ch, p1if, p1og, c1t[ch], h1b[ch], slice(0, 64))
                    # pred reuses a dead region of p1if (already consumed)
                    pp = p1if[0:4, 0:CW]
                    nc.tensor.matmul(pp, wfcb[:], h1b[ch][:], start=True, stop=True)
                    po = tpool.tile([4, CW], f32, tag=f"po{ch}")
                    nc.vector.tensor_copy(po[:], pp)
                    nc.sync.dma_start(out_d.ap()[s - T_, :, cs], po[:])
                    if not last:
                        p0if, p0og = ar_mms(ch, w0h0, w0h1b)
                        ar_tail(ch, p0if, p0og, c0t[ch], h0t[ch], slice(0, 64))

    nc.compile()
    return nc


def _prep_inputs(x, Wih0, Whh0, bih0, bhh0, Wih1, Whh1, bih1, bhh1, Wfc, bfc,
                 T_, STEPS_):
    """Host-side layout prep. Returns (shared_weight_arrays, per_core_xt)."""
    f = np.float32
    h16 = np.float16
    x = np.asarray(x, f)
    Wih0, Whh0 = np.asarray(Wih0, f), np.asarray(Whh0, f)
    Wih1, Whh1 = np.asarray(Wih1, f), np.asarray(Whh1, f)
    Wfc = np.asarray(Wfc, f)
    b0 = np.asarray(bih0, f) + np.asarray(bhh0, f)   # [4H]
    b1 = np.asarray(bih1, f) + np.asarray(bhh1, f)
    bfc = np.asarray(bfc, f)

    def gate(Wm, q):
        return Wm[q * H:(q + 1) * H]

    # encode block lhsT [128k, 4 gates x 128m]
    wencblk = np.zeros((128, 512), f)
    wencx = np.zeros((128, 512), f)
    for q in range(4):
        blk = np.zeros((128, 128), f)
        blk[0:64, 0:64] = gate(Whh0, q).T
        blk[0:64, 64:128] = gate(Wih1, q).T
        blk[64:128, 64:128] = gate(Whh1, q).T
        wencblk[:, q * 128:(q + 1) * 128] = blk
        wx = np.zeros((5, 128), f)
        wx[0:4, 0:64] = gate(Wih0, q).T
        wx[4, 0:64] = gate(b0[:, None], q)[:, 0]
        wx[4, 64:128] = gate(b1[:, None], q)[:, 0]
        wencx[0:5, q * 128:(q + 1) * 128] = wx

    # AR weights: lhsT col blocks = gates in order (i, f, o, g), M=64 each
    Wcomb = Wih0 @ Wfc          # [4H, 64]
    b0p = b0 + Wih0 @ bfc
    colq = (0, 1, 3, 2)
    w1h0 = np.zeros((65, 256), f)
    w1h1b = np.zeros((65, 256), f)
    w0h0 = np.zeros((65, 256), f)
    w0h1b = np.zeros((65, 256), f)
    for j, q in enumerate(colq):
        mc = slice(j * 64, (j + 1) * 64)
        w1h0[0:64, mc] = gate(Wih1, q).T
        w1h1b[0:64, mc] = gate(Whh1, q).T
        w1h1b[64, mc] = gate(b1[:, None], q)[:, 0]
        w0h0[0:64, mc] = gate(Whh0, q).T
        w0h1b[0:64, mc] = gate(Wcomb, q).T
        w0h1b[64, mc] = gate(b0p[:, None], q)[:, 0]
    wfcb = np.zeros((65, 4), f)
    wfcb[0:64] = Wfc.T
    wfcb[64] = bfc

    shared = dict(wencblk=wencblk.astype(h16), wencx=wencx.astype(h16),
                  w1h0=w1h0.astype(h16), w1h1b=w1h1b.astype(h16),
                  w0h0=w0h0.astype(h16), w0h1b=w0h1b.astype(h16),
                  wfcb=wfcb.astype(h16),
                  ones_row=np.ones((1, BC), h16))

    # per-core x-tilde: [T+1, 5, BC]; row 4 = ones; step T duplicates x_{T-1}
    xts = []
    for c in range(NCORES):
        xs = x[c * BC:(c + 1) * BC, :T_, :]          # [BC, T_, D]
        xt = np.ones((T_ + 1, 5, BC), f)
        xt[:T_, 0:4, :] = np.transpose(xs, (1, 2, 0))
        xt[T_, 0:4, :] = xs[:, T_ - 1, :].T
        xts.append(xt.astype(h16))
    return shared, xts


def kernel(**inputs):
    return _run(T, STEPS, **inputs)


def _run(T_, STEPS_, x, Wih0, Whh0, bih0, bhh0, Wih1, Whh1, bih1, bhh1,
         Wfc, bfc):
    from concourse.bass_utils import run_bass_kernel_spmd

    key = (T_, STEPS_)
    if key not in _cache:
        _cache[key] = _build(T_, STEPS_)
    nc = _cache[key]

    shared, xts = _prep_inputs(x, Wih0, Whh0, bih0, bhh0, Wih1, Whh1,
                               bih1, bhh1, Wfc, bfc, T_, STEPS_)
    in_maps = [{**shared, "xt": xts[c]} for c in range(NCORES)]
    res = run_bass_kernel_spmd(nc, in_maps, core_ids=list(range(NCORES)),
                               trace=TRACE)
    LAST["exec_time_ns"] = res.exec_time_ns
    LAST["res"] = res
    out = np.empty((B, STEPS_, 4), np.float32)
    for c in range(NCORES):
        # res: [STEPS, 4, BC] -> [BC, STEPS, 4]
        out[c * BC:(c + 1) * BC] = np.transpose(res.results[c]["out"], (2, 0, 1))
    return out
